# revision 15
# baseline (speedup 1.0000x reference)
"""GRU kernel for Trainium2, 8-way batch data-parallel.

Problem: flow_x [64, 1024, 512, 1] -> GRU over T=512 steps, hidden=1024,
returns final hidden state [64, 1024, 1, 1].

Strategy per core (8 batches each, no collectives):
  - T-layout: hidden index n = j*128 + p lives at [partition p, slice j].
    The hidden state is the matmul moving operand directly; elementwise
    ops run at 128-partition utilization; no per-step transposes.
  - Recurrence matmul: out.T = W_tile.T @ h.T with W tiles [128,128]
    stationary in bf16 (FWL weight loads) and h.T [128, 8] moving.
    24 m-tiles x 8 k-tiles = 192 LDW+MM pairs per step -- the PE
    weight-load port is the hard floor (~5.2us/step).
  - The per-step gate chain (sigmoid/tanh/blend) runs as 4 pair-chains
    of [128, 2, 8] pipelined against the PE stream; h is kept as 4
    pair-tiles, ping-pong buffered per step parity.
  - Input projections x@W1+b1 / x@W2+b2 for chunk c+1 are interleaved
    into chunk c's recurrence one m-tile group per step: the projection
    matmuls fill the PE bubble at each step boundary (waiting for the
    last h pair-chain), instead of forming a serial phase.  Projection
    outputs ping-pong between two buffers (loop body = 2 chunks).
  - flow_x / W1 / W2 are cast to bf16 on the host: halves transfer
    bytes and removes on-chip conversion stages.
"""

import sys

sys.path.insert(0, "/opt/trn_rl_repo")

import numpy as np

B, N, T = 64, 1024, 512
NCORES = 8
BLOC = B // NCORES  # 8 batches per core
TC = 32  # timestep chunk
NCHUNK = T // TC  # 16
KT = N // 128  # 8 k-tiles
NJ = 8  # hidden slices of 128
NP = NJ // 2  # 4 pair-chains per step

_CACHE = {}


def _build_nc(n_chunks=NCHUNK, tc_steps=TC):
    """Build + compile the bass program. Parameterized so a mini version
    can be checked in CoreSim (n_chunks must be 1 or even)."""
    from contextlib import ExitStack

    import concourse.bacc as bacc
    import concourse.bass as bass
    import concourse.mybir as mybir
    import concourse.tile as tile

    f32 = mybir.dt.float32
    bf16 = mybir.dt.bfloat16
    AF = mybir.ActivationFunctionType

    nc = bacc.Bacc("TRN2", target_bir_lowering=False, debug=False)

    fx = nc.dram_tensor("flow_x", [BLOC, N, T], bf16, kind="ExternalInput")
    w1 = nc.dram_tensor("W1", [N, 2 * N], bf16, kind="ExternalInput")
    b1 = nc.dram_tensor("b1", [2 * N], f32, kind="ExternalInput")
    w2 = nc.dram_tensor("W2", [N, N], bf16, kind="ExternalInput")
    b2 = nc.dram_tensor("b2", [N], f32, kind="ExternalInput")
    out = nc.dram_tensor("h_out", [BLOC, N], f32, kind="ExternalOutput")

    NPROJ = 3 * NJ  # 24 projection m-tile groups per chunk

    with tile.TileContext(nc) as tc:
        with ExitStack() as stk:
            const = stk.enter_context(tc.tile_pool(name="const", bufs=1))
            sb = stk.enter_context(tc.tile_pool(name="sb", bufs=3))
            ppool = stk.enter_context(
                tc.tile_pool(name="pp", bufs=4, space=bass.MemorySpace.PSUM)
            )
            projp = stk.enter_context(
                tc.tile_pool(name="projp", bufs=2, space=bass.MemorySpace.PSUM)
            )

            # ---- weights resident in SBUF (bf16, straight DMA) ----
            w1b = const.tile([128, KT, 2 * N], bf16, tag="w1b")
            nc.sync.dma_start(w1b[:], w1[:].rearrange("(k p) c -> p k c", p=128))
            w2b = const.tile([128, KT, N], bf16, tag="w2b")
            nc.sync.dma_start(w2b[:], w2[:].rearrange("(k p) c -> p k c", p=128))

            # biases as [128, slice] columns; b1 applied twice (reference
            # adds b1 in both the x- and h-projections).
            b1s = const.tile([128, 16], f32, tag="b1s")
            nc.sync.dma_start(b1s[:], b1[:].rearrange("(s p) -> p s", p=128))
            b1d = const.tile([128, 16], f32, tag="b1d")
            nc.vector.tensor_scalar_mul(b1d[:], b1s[:], 2.0)
            b2s = const.tile([128, 8], f32, tag="b2s")
            nc.sync.dma_start(b2s[:], b2[:].rearrange("(s p) -> p s", p=128))

            # ---- persistent buffers ----
            # chunk input staging (one chunk of flow_x, prefetched)
            xinb = const.tile([128, KT, BLOC, tc_steps], bf16, tag="xinb")
            # projection outputs, A/B ping-pong across chunks
            xzr = [
                const.tile(
                    [128, NP, 2, 2, BLOC, tc_steps],
                    bf16,
                    name=f"xzr{a}",
                    tag=f"xzr{a}",
                )
                for a in range(2)
            ]
            xh = [
                const.tile(
                    [128, NP, 2, BLOC, tc_steps], bf16, name=f"xh{a}", tag=f"xh{a}"
                )
                for a in range(2)
            ]
            # hidden state: 4 pair-tiles [128, 2, B], ping-pong per step
            h32 = [
                [
                    const.tile(
                        [128, 2, BLOC], f32, name=f"h32_{a}_{j}", tag=f"h32_{a}_{j}"
                    )
                    for j in range(NP)
                ]
                for a in range(2)
            ]
            htb = [
                [
                    const.tile(
                        [128, 2, BLOC], bf16, name=f"htb_{a}_{j}", tag=f"htb_{a}_{j}"
                    )
                    for j in range(NP)
                ]
                for a in range(2)
            ]
            for j in range(NP):
                nc.vector.memset(h32[0][j][:], 0.0)
                nc.vector.memset(htb[0][j][:], 0.0)

            def wz(j, k):
                return w1b[:, k, j * 128 : (j + 1) * 128]

            def wr(j, k):
                return w1b[:, k, N + j * 128 : N + (j + 1) * 128]

            def wh(j, k):
                return w2b[:, k, j * 128 : (j + 1) * 128]

            def dma_xin(col_slice):
                for k in range(KT):
                    nc.sync.dma_start(
                        xinb[:, k],
                        fx[:, k * 128 : (k + 1) * 128, col_slice].rearrange(
                            "b p t -> p b t"
                        ),
                    )

            def proj_group(m, pbuf):
                """One projection m-tile group (8 MMs + biased copy) for the
                NEXT chunk, reading xinb, writing xzr/xh buffer `pbuf`."""
                j, g = m // 3, m % 3
                jj, sub = j // 2, j % 2
                wt, dst, bias = (
                    (wz, xzr[pbuf][:, jj, 0, sub], b1d[:, j : j + 1]),
                    (wr, xzr[pbuf][:, jj, 1, sub], b1d[:, 8 + j : 9 + j]),
                    (wh, xh[pbuf][:, jj, sub], b2s[:, j : j + 1]),
                )[g]
                pj = projp.tile([128, BLOC, tc_steps], f32, tag="pj")
                for k in range(KT):
                    nc.tensor.matmul(
                        pj[:], wt(j, k), xinb[:, k], start=(k == 0), stop=(k == KT - 1)
                    )
                nc.scalar.activation(dst, pj[:], AF.Identity, bias=bias)

            def rec_step(t, rbuf, pbuf):
                """One GRU step reading projections from buffer `rbuf`;
                interleaves one next-chunk projection group per step when
                pbuf is not None."""
                src, dst = t % 2, 1 - t % 2

                def hmov(k):
                    return htb[src][k // 2][:, k % 2, :]

                for jj in range(NP):
                    # psum [p, gate(z,r,h), sub, b]; gate order hh,z,r so
                    # the hh-path (s) is ready before the pair's PE span ends
                    pp = ppool.tile([128, 3, 2, BLOC], f32, tag="pp")
                    for g, wt in ((2, wh), (0, wz), (1, wr)):
                        for sub in range(2):
                            j = 2 * jj + sub
                            for k in range(KT):
                                nc.tensor.matmul(
                                    pp[:, g, sub, :],
                                    wt(j, k),
                                    hmov(k),
                                    start=(k == 0),
                                    stop=(k == KT - 1),
                                )
                    # s = h@W2 + b2 per sub (ScalarE: PSUM read + bias)
                    s = sb.tile([128, 2, BLOC], f32, tag="s")
                    for sub in range(2):
                        nc.scalar.activation(
                            s[:, sub],
                            pp[:, 2, sub, :],
                            AF.Identity,
                            bias=b2s[:, 2 * jj + sub : 2 * jj + sub + 1],
                        )
                    tzr = sb.tile([128, 2, 2, BLOC], f32, tag="tzr")
                    nc.vector.tensor_add(
                        tzr[:], pp[:, 0:2, :, :], xzr[rbuf][:, jj, :, :, :, t]
                    )
                    zr = sb.tile([128, 2, 2, BLOC], f32, tag="zr")
                    nc.scalar.activation(zr[:], tzr[:], AF.Sigmoid)
                    # z-side products (DVE, off critical)
                    wm = sb.tile([128, 2, BLOC], f32, tag="wm")
                    nc.vector.tensor_scalar(
                        out=wm[:],
                        in0=zr[:, 0],
                        scalar1=-1.0,
                        scalar2=1.0,
                        op0=mybir.AluOpType.mult,
                        op1=mybir.AluOpType.add,
                    )
                    m1 = sb.tile([128, 2, BLOC], f32, tag="m1")
                    nc.vector.tensor_mul(m1[:], zr[:, 0], h32[src][jj][:])
                    # h_hat path
                    s2 = sb.tile([128, 2, BLOC], f32, tag="s2")
                    nc.vector.tensor_mul(s2[:], zr[:, 1], s[:])
                    s3 = sb.tile([128, 2, BLOC], f32, tag="s3")
                    nc.vector.tensor_add(s3[:], s2[:], xh[rbuf][:, jj, :, :, t])
                    hh = sb.tile([128, 2, BLOC], f32, tag="hh")
                    nc.scalar.activation(hh[:], s3[:], AF.Tanh)
                    # h_new = z*h + (1-z)*hh ; bf16 write first (critical)
                    m2 = sb.tile([128, 2, BLOC], f32, tag="m2")
                    nc.vector.tensor_mul(m2[:], wm[:], hh[:])
                    nc.vector.tensor_add(htb[dst][jj][:], m1[:], m2[:])
                    # lazy f32 master copy on Pool (read next step by m1)
                    nc.gpsimd.tensor_add(h32[dst][jj][:], m1[:], m2[:])

                # next-chunk projection rides the step-boundary PE bubble
                if pbuf is not None and t < NPROJ:
                    proj_group(t, pbuf)

            def emit_chunk(rbuf, pbuf, prefetch_slice):
                """Recurrence for one chunk (reading rbuf); projections for
                the next chunk (into pbuf) interleaved; then prefetch the
                chunk after that into xinb."""
                for t in range(tc_steps):
                    rec_step(t, rbuf, pbuf)
                if prefetch_slice is not None:
                    dma_xin(prefetch_slice)

            if n_chunks > 1:
                assert tc_steps >= NPROJ, "need tc_steps >= 24 to absorb projections"

            # ---- prologue: chunk 0 projections (serial) ----
            dma_xin(bass.ds(0, tc_steps))
            for m in range(NPROJ):
                proj_group(m, 0)
            if n_chunks > 1:
                assert n_chunks % 2 == 0 and tc_steps % 2 == 0
                dma_xin(bass.ds(tc_steps, tc_steps))

                niter = n_chunks // 2 - 1
                if niter > 0:
                    with tc.For_i(0, niter, 1) as i:
                        c2 = 2 * tc_steps
                        emit_chunk(0, 1, bass.ds(i * c2 + 2 * tc_steps, tc_steps))
                        emit_chunk(1, 0, bass.ds(i * c2 + 3 * tc_steps, tc_steps))
                # epilogue: last two chunks
                emit_chunk(0, 1, None)
                emit_chunk(1, None, None)
            else:
                emit_chunk(0, None, None)

            # final h -> DRAM: out[b, (2*jj+sub)*128+p] = h32[fin][jj][p, sub, b]
            fin = tc_steps % 2
            for jj in range(NP):
                for sub in range(2):
                    j = 2 * jj + sub
                    nc.sync.dma_start(
                        out[:, j * 128 : (j + 1) * 128].rearrange("b p -> p b"),
                        h32[fin][jj][:, sub, :],
                    )

    nc.compile()
    return nc


def kernel(flow_x, W1, b1, W2, b2):
    import ml_dtypes

    from concourse.bass_utils import run_bass_kernel_spmd

    if "nc" not in _CACHE:
        _CACHE["nc"] = _build_nc()
    nc = _CACHE["nc"]

    bf = ml_dtypes.bfloat16
    fx = np.ascontiguousarray(flow_x.reshape(B, N, T).astype(np.float32).astype(bf))
    w1b = np.ascontiguousarray(np.asarray(W1, np.float32).astype(bf))
    w2b = np.ascontiguousarray(np.asarray(W2, np.float32).astype(bf))
    b1f = np.ascontiguousarray(np.asarray(b1, np.float32))
    b2f = np.ascontiguousarray(np.asarray(b2, np.float32))
    in_maps = []
    for c in range(NCORES):
        in_maps.append(
            {
                "flow_x": fx[c * BLOC : (c + 1) * BLOC],
                "W1": w1b,
                "b1": b1f,
                "W2": w2b,
                "b2": b2f,
            }
        )
    res = run_bass_kernel_spmd(nc, in_maps, list(range(NCORES)))
    outs = [res.results[c]["h_out"] for c in range(NCORES)]
    h = np.concatenate(outs, axis=0)
    return h.reshape(B, N, 1, 1).astype(np.float32)
